# revision 9
# baseline (speedup 1.0000x reference)
"""GraphSAGE on 8 TRN2 cores — sharded-h1, per-slot gathers, Q7-lean.

Structure (sharded-h1): layer-1 h1 is computed once globally at the ~75K
nodes referenced by the output, sharded over 8 cores; shards are
AllGathered into a Shared-scratchpad h1 table; each core then runs layer 2
+ projection for its 1024 batch rows.

HW facts driving this version (probed on-device):
- An indirect DMA consumes exactly ONE offset per output partition; larger
  offset APs silently gather consecutive rows (v2's bug). So neighbor
  gathers are per-(tile,slot) instructions of 128 rows each, and the
  binding resource is the Q7 SWDGE fixed cost (~0.93us/instruction).
- dma_gather (int16) cannot address the 100K/77K-row tables.
Therefore the wins here are instruction-count and tail wins:
- own-node features come in pre-transposed per-core shards (xot input,
  the data-parallel shard of x), killing 1-of-17 gather instructions and
  the per-tile own transpose+copy;
- gather index tables are preloaded to SBUF once;
- h1_full is a per-core (Local) replica: a Shared-scratchpad variant
  concentrated all 8 cores' phase-2 gather reads on one HBM region and
  ran phase-2 gathers ~5x slower. The AllGather is chunked geometrically
  (50/30/17/3% of rows) so all but the last ~3% overlaps phase-1 work.
"""

import numpy as np

import concourse.bacc as bacc
import concourse.bass as bass
import concourse.mybir as mybir
import concourse.tile as tile
from concourse.bass_utils import run_bass_kernel_spmd
from concourse.masks import make_identity
from contextlib import ExitStack

N_NODES = 100000
D = 128
S = 16
BATCH = 8192
N_CORES = 8
NB = BATCH // N_CORES            # 1024 batch rows per core
P = 128
SELF_TILES = NB // P             # 8 phase-2 tiles per core
GB = 4                           # phase-1 tiles per wide buffer
CHUNK_FRACS = (0.5, 0.8, 0.97)  # AllGather chunk boundaries (tile fracs)

F32 = mybir.dt.float32
I32 = mybir.dt.int32


def _chunk_tiles(n_tiles1):
    cb = [0] + [max(1, round(f * n_tiles1)) for f in CHUNK_FRACS] + [n_tiles1]
    # ensure strictly increasing
    for i in range(1, len(cb)):
        cb[i] = max(cb[i], cb[i - 1] + 1) if cb[i - 1] + 1 <= n_tiles1 else cb[i - 1]
    cb[-1] = n_tiles1
    return sorted(set(cb))


def build_program(n_tiles1):
    mp8 = n_tiles1 * P
    mp = mp8 * N_CORES
    cb = _chunk_tiles(n_tiles1)
    nc = bacc.Bacc("TRN2", target_bir_lowering=False, debug=False,
                   enable_asserts=False, num_devices=N_CORES)

    x_d = nc.dram_tensor("x", [N_NODES, D], F32, kind="ExternalInput").ap()
    xot_d = nc.dram_tensor("xot", [P, mp8], F32, kind="ExternalInput").ap()
    g1_d = nc.dram_tensor("g1", [P, n_tiles1 * S], I32,
                          kind="ExternalInput").ap()
    g2_d = nc.dram_tensor("g2", [P, SELF_TILES * (S + 1)], I32,
                          kind="ExternalInput").ap()
    w1a_d = nc.dram_tensor("w1a", [P, P], F32, kind="ExternalInput").ap()
    w1b_d = nc.dram_tensor("w1b", [P, P], F32, kind="ExternalInput").ap()
    w2a_d = nc.dram_tensor("w2a", [P, P], F32, kind="ExternalInput").ap()
    w2b_d = nc.dram_tensor("w2b", [P, P], F32, kind="ExternalInput").ap()
    wout_d = nc.dram_tensor("wout", [P, 64], F32, kind="ExternalInput").ap()
    b1_d = nc.dram_tensor("b1", [P, 1], F32, kind="ExternalInput").ap()
    b2_d = nc.dram_tensor("b2", [P, 1], F32, kind="ExternalInput").ap()
    bout_d = nc.dram_tensor("bout", [64, 1], F32, kind="ExternalInput").ap()
    out_d = nc.dram_tensor("out", [NB, 64], F32, kind="ExternalOutput").ap()

    h1_mine = nc.dram_tensor("h1_mine", [mp8, D], F32)
    h1_full = nc.dram_tensor("h1_full", [mp, D], F32)

    with tile.TileContext(nc) as tc, ExitStack() as ctx:
        consts = ctx.enter_context(tc.tile_pool(name="consts", bufs=1))
        wide_pool = ctx.enter_context(tc.tile_pool(name="wide", bufs=2))
        sb_pool = ctx.enter_context(tc.tile_pool(name="sb", bufs=3))
        ps = ctx.enter_context(tc.tile_pool(name="ps", bufs=2, space="PSUM"))

        w1a = consts.tile([P, P], F32, tag="w1a")
        w1b = consts.tile([P, P], F32, tag="w1b")
        w2a = consts.tile([P, P], F32, tag="w2a")
        w2b = consts.tile([P, P], F32, tag="w2b")
        wout = consts.tile([P, 64], F32, tag="wout")
        b1 = consts.tile([P, 1], F32, tag="b1")
        b2 = consts.tile([P, 1], F32, tag="b2")
        bout = consts.tile([64, 1], F32, tag="bout")
        ident = consts.tile([P, P], F32, tag="ident")
        xot = consts.tile([P, mp8], F32, tag="xot")
        g1 = consts.tile([P, n_tiles1 * S], I32, tag="g1")
        g2 = consts.tile([P, SELF_TILES * (S + 1)], I32, tag="g2")
        for t_sb, t_d in ((g1, g1_d), (w1a, w1a_d), (w1b, w1b_d),
                          (w2a, w2a_d), (w2b, w2b_d), (wout, wout_d),
                          (b1, b1_d), (b2, b2_d), (bout, bout_d),
                          (xot, xot_d), (g2, g2_d)):
            nc.sync.dma_start(out=t_sb[:], in_=t_d[:, :])
        make_identity(nc, ident[:])

        def neigh_aggT(wide, n_slots):
            """max-agg the gathered neighbor rows and return aggT (feature-
            major) in SBUF."""
            agg = sb_pool.tile([P, D], F32, tag="agg")
            wide3 = wide[:].rearrange("p (s f) -> p f s", f=D)
            nc.vector.reduce_max(agg[:], wide3[:, :, (n_slots - S):],
                                 axis=mybir.AxisListType.X)
            aggT_ps = ps.tile([P, P], F32, tag="aggT_ps", space="PSUM")
            nc.tensor.transpose(aggT_ps[:], agg[:], ident[:])
            aggT = sb_pool.tile([P, P], F32, tag="aggT")
            nc.scalar.copy(out=aggT[:], in_=aggT_ps[:])
            return aggT

        def h_from(ownT_ap, aggT, wa, wb, bias, act_fn):
            hps = ps.tile([P, P], F32, tag="hps", space="PSUM")
            nc.tensor.matmul(hps[:], lhsT=wa[:], rhs=ownT_ap, start=True,
                             stop=False)
            nc.tensor.matmul(hps[:], lhsT=wb[:], rhs=aggT[:], start=False,
                             stop=True)
            hT = sb_pool.tile([P, P], F32, tag="hT")
            nc.scalar.activation(hT[:], hps[:], act_fn, bias=bias[:, :1])
            return hT

        # ---- phase 1: h1 for this core's shard (own rows from xot) ----
        ci = 0
        for b in range(n_tiles1 // GB):
            wide = wide_pool.tile([P, GB * S * D], F32, tag="wide")
            for s in range(GB * S):
                nc.gpsimd.indirect_dma_start(
                    out=wide[:, s * D:(s + 1) * D],
                    out_offset=None,
                    in_=x_d[:, :],
                    in_offset=bass.IndirectOffsetOnAxis(
                        ap=g1[:, b * GB * S + s:b * GB * S + s + 1], axis=0),
                )
            wide3 = wide[:].rearrange("p (x f) -> p f x", f=D)
            for i in range(GB):
                t = b * GB + i
                agg = sb_pool.tile([P, D], F32, tag="agg")
                nc.vector.reduce_max(agg[:], wide3[:, :, i * S:(i + 1) * S],
                                     axis=mybir.AxisListType.X)
                aggT_ps = ps.tile([P, P], F32, tag="aggT_ps", space="PSUM")
                nc.tensor.transpose(aggT_ps[:], agg[:], ident[:])
                aggT = sb_pool.tile([P, P], F32, tag="aggT")
                nc.scalar.copy(out=aggT[:], in_=aggT_ps[:])
                hT = h_from(xot[:, t * P:(t + 1) * P], aggT, w1a, w1b, b1,
                            mybir.ActivationFunctionType.Relu)
                nm_ps = ps.tile([P, P], F32, tag="nm_ps", space="PSUM")
                nc.tensor.transpose(nm_ps[:], hT[:], ident[:])
                h1nm = sb_pool.tile([P, P], F32, tag="h1nm")
                nc.vector.tensor_copy(out=h1nm[:], in_=nm_ps[:])
                nc.sync.dma_start(out=h1_mine.ap()[t * P:(t + 1) * P, :],
                                  in_=h1nm[:])
                if t + 1 == cb[ci + 1]:
                    r0, r1 = cb[ci] * P, cb[ci + 1] * P
                    nc.gpsimd.collective_compute(
                        "AllGather",
                        mybir.AluOpType.bypass,
                        replica_groups=[list(range(N_CORES))],
                        ins=[h1_mine.ap()[r0:r1, :].opt()],
                        outs=[h1_full.ap()[r0 * N_CORES:r1 * N_CORES, :].opt()],
                    )
                    ci += 1

        # ---- phase 2: layer 2 + projection for this core's batch rows ----
        for g in range(SELF_TILES):
            wide = wide_pool.tile([P, (S + 1) * D], F32, tag="wide2")
            for s in range(S + 1):
                nc.gpsimd.indirect_dma_start(
                    out=wide[:, s * D:(s + 1) * D],
                    out_offset=None,
                    in_=h1_full.ap()[:, :],
                    in_offset=bass.IndirectOffsetOnAxis(
                        ap=g2[:, g * (S + 1) + s:g * (S + 1) + s + 1], axis=0),
                )
            aggT = neigh_aggT(wide, S + 1)
            ownT_ps = ps.tile([P, P], F32, tag="nm_ps", space="PSUM")
            nc.tensor.transpose(ownT_ps[:], wide[:, 0:D], ident[:])
            ownT = sb_pool.tile([P, P], F32, tag="ownT")
            nc.scalar.copy(out=ownT[:], in_=ownT_ps[:])
            h2T = h_from(ownT[:], aggT, w2a, w2b, b2,
                         mybir.ActivationFunctionType.Identity)
            ops = ps.tile([64, P], F32, tag="hps", space="PSUM")
            nc.tensor.matmul(ops[:], lhsT=wout[:], rhs=h2T[:], start=True,
                             stop=True)
            outT = sb_pool.tile([64, P], F32, tag="outT")
            nc.scalar.activation(outT[:], ops[:],
                                 mybir.ActivationFunctionType.Identity,
                                 bias=bout[:, :1])
            trps = ps.tile([P, 64], F32, tag="aggT_ps", space="PSUM")
            nc.tensor.transpose(trps[:], outT[:], ident[:64, :64])
            outsb = sb_pool.tile([P, 64], F32, tag="outsb")
            nc.vector.tensor_copy(out=outsb[:], in_=trps[:])
            nc.sync.dma_start(out=out_d[g * P:(g + 1) * P, :], in_=outsb[:])

    return nc


_CACHE = {}


def _get_compiled(n_tiles1):
    if _CACHE.get("n_tiles1") != n_tiles1:
        nc = build_program(n_tiles1)
        nc.compile()
        _CACHE["nc"] = nc
        _CACHE["n_tiles1"] = n_tiles1
    return _CACHE["nc"]


def make_in_maps(x, neigh_idx, batch, W1, b1, W2, b2, Wout, bout):
    x = np.ascontiguousarray(np.asarray(x, dtype=np.float32))
    ni = np.asarray(neigh_idx, dtype=np.int64)
    bt = np.asarray(batch, dtype=np.int64)
    W1 = np.asarray(W1, dtype=np.float32)
    W2 = np.asarray(W2, dtype=np.float32)

    ref_nodes = np.unique(np.concatenate([bt, ni[bt].ravel()]))
    M = len(ref_nodes)
    mp8 = -(-M // (N_CORES * P * GB)) * (P * GB)
    mp = mp8 * N_CORES
    n_tiles1 = mp8 // P
    padded = np.concatenate([ref_nodes,
                             np.zeros(mp - M, dtype=ref_nodes.dtype)])

    cb = np.array(_chunk_tiles(n_tiles1)) * P       # row chunk boundaries
    pos = np.arange(mp)
    c, r = pos // mp8, pos % mp8
    k = np.searchsorted(cb, r, side="right") - 1     # chunk of each row
    szk = (cb[k + 1] - cb[k])
    full_pos = N_CORES * cb[k] + c * szk + (r - cb[k])
    glob2loc = np.zeros(N_NODES, dtype=np.int64)
    glob2loc[ref_nodes] = full_pos[:M]

    common = {
        "x": x,
        "w1a": np.ascontiguousarray(W1[:P]),
        "w1b": np.ascontiguousarray(W1[P:]),
        "w2a": np.ascontiguousarray(W2[:P]),
        "w2b": np.ascontiguousarray(W2[P:]),
        "wout": np.ascontiguousarray(np.asarray(Wout, np.float32)),
        "b1": np.ascontiguousarray(np.asarray(b1, np.float32).reshape(P, 1)),
        "b2": np.ascontiguousarray(np.asarray(b2, np.float32).reshape(P, 1)),
        "bout": np.ascontiguousarray(np.asarray(bout, np.float32).reshape(64, 1)),
    }
    in_maps = []
    for cc in range(N_CORES):
        own1 = padded[cc * mp8:(cc + 1) * mp8]
        # xot: this core's own-node features, feature-major [128, mp8]
        xot = np.ascontiguousarray(x[own1].T)
        # g1: [128, T*S] with g1[p, t*S+s] = ni[own1[t*128+p], s]
        n1 = ni[own1].reshape(n_tiles1, P, S).transpose(1, 0, 2)
        g1 = np.ascontiguousarray(n1.reshape(P, n_tiles1 * S).astype(np.int32))
        bc = bt[cc * NB:(cc + 1) * NB]
        l2 = np.concatenate([glob2loc[bc][:, None], glob2loc[ni[bc]]], axis=1)
        l2 = l2.reshape(SELF_TILES, P, S + 1).transpose(1, 0, 2)
        g2 = np.ascontiguousarray(
            l2.reshape(P, SELF_TILES * (S + 1)).astype(np.int32))
        in_maps.append(dict(common, xot=xot, g1=g1, g2=g2))
    return in_maps, n_tiles1


def run(in_maps, n_tiles1, trace=False, **kw):
    nc = _get_compiled(n_tiles1)
    return run_bass_kernel_spmd(nc, in_maps, core_ids=list(range(N_CORES)),
                                trace=trace, **kw)


def kernel(x, neigh_idx, batch, W1, b1, W2, b2, Wout, bout):
    in_maps, n_tiles1 = make_in_maps(x, neigh_idx, batch, W1, b1, W2, b2,
                                     Wout, bout)
    res = run(in_maps, n_tiles1)
    outs = [np.asarray(res.results[c]["out"]) for c in range(N_CORES)]
    return np.concatenate(outs, axis=0).astype(np.float32)


# revision 12
# speedup vs baseline: 1.0353x; 1.0353x over previous
"""GraphSAGE on 8 TRN2 cores — sharded-h1, per-slot gathers, Q7-lean.

Structure (sharded-h1): layer-1 h1 is computed once globally at the ~75K
nodes referenced by the output, sharded over 8 cores; shards are
AllGathered into a Shared-scratchpad h1 table; each core then runs layer 2
+ projection for its 1024 batch rows.

HW facts driving this version (probed on-device):
- An indirect DMA consumes exactly ONE offset per output partition; larger
  offset APs silently gather consecutive rows (v2's bug). So neighbor
  gathers are per-(tile,slot) instructions of 128 rows each, and the
  binding resource is the Q7 SWDGE fixed cost (~0.93us/instruction).
- dma_gather (int16) cannot address the 100K/77K-row tables.
Therefore the wins here are instruction-count and tail wins:
- own-node features come in pre-transposed per-core shards (xot input,
  the data-parallel shard of x), killing 1-of-17 gather instructions and
  the per-tile own transpose+copy;
- gather index tables are preloaded to SBUF once;
- h1_full is a per-core (Local) replica: a Shared-scratchpad variant
  concentrated all 8 cores' phase-2 gather reads on one HBM region and
  ran phase-2 gathers ~5x slower. The AllGather is chunked geometrically
  (50/30/17/3% of rows) so all but the last ~3% overlaps phase-1 work.
"""

import numpy as np

import concourse.bacc as bacc
import concourse.bass as bass
import concourse.mybir as mybir
import concourse.tile as tile
from concourse.bass_utils import run_bass_kernel_spmd
from concourse.masks import make_identity
from contextlib import ExitStack

N_NODES = 100000
D = 128
S = 16
BATCH = 8192
N_CORES = 8
NB = BATCH // N_CORES            # 1024 batch rows per core
P = 128
SELF_TILES = NB // P             # 8 phase-2 tiles per core
CHUNK_FRACS = (0.5, 0.8, 0.97)  # AllGather chunk boundaries (tile fracs)

F32 = mybir.dt.float32
I32 = mybir.dt.int32


def _chunk_tiles(n_tiles1):
    cb = [0] + [max(1, round(f * n_tiles1)) for f in CHUNK_FRACS] + [n_tiles1]
    # ensure strictly increasing
    for i in range(1, len(cb)):
        cb[i] = max(cb[i], cb[i - 1] + 1) if cb[i - 1] + 1 <= n_tiles1 else cb[i - 1]
    cb[-1] = n_tiles1
    return sorted(set(cb))


def build_program(n_tiles1):
    mp8 = n_tiles1 * P
    mp = mp8 * N_CORES
    cb = _chunk_tiles(n_tiles1)
    nc = bacc.Bacc("TRN2", target_bir_lowering=False, debug=False,
                   enable_asserts=False, num_devices=N_CORES)

    x_d = nc.dram_tensor("x", [N_NODES, D], F32, kind="ExternalInput").ap()
    xot_d = nc.dram_tensor("xot", [P, mp8], F32, kind="ExternalInput").ap()
    g1_d = nc.dram_tensor("g1", [P, n_tiles1 * S], I32,
                          kind="ExternalInput").ap()
    g2_d = nc.dram_tensor("g2", [P, SELF_TILES * (S + 1)], I32,
                          kind="ExternalInput").ap()
    w1a_d = nc.dram_tensor("w1a", [P, P], F32, kind="ExternalInput").ap()
    w1b_d = nc.dram_tensor("w1b", [P, P], F32, kind="ExternalInput").ap()
    w2a_d = nc.dram_tensor("w2a", [P, P], F32, kind="ExternalInput").ap()
    w2b_d = nc.dram_tensor("w2b", [P, P], F32, kind="ExternalInput").ap()
    wout_d = nc.dram_tensor("wout", [P, 64], F32, kind="ExternalInput").ap()
    b1_d = nc.dram_tensor("b1", [P, 1], F32, kind="ExternalInput").ap()
    b2_d = nc.dram_tensor("b2", [P, 1], F32, kind="ExternalInput").ap()
    bout_d = nc.dram_tensor("bout", [64, 1], F32, kind="ExternalInput").ap()
    out_d = nc.dram_tensor("out", [NB, 64], F32, kind="ExternalOutput").ap()

    h1_mine = nc.dram_tensor("h1_mine", [mp8, D], F32)
    h1_full = nc.dram_tensor("h1_full", [mp, D], F32)

    with tile.TileContext(nc) as tc, ExitStack() as ctx:
        consts = ctx.enter_context(tc.tile_pool(name="consts", bufs=1))
        wide_pool = ctx.enter_context(tc.tile_pool(name="wide", bufs=6))
        sb_pool = ctx.enter_context(tc.tile_pool(name="sb", bufs=3))
        ps = ctx.enter_context(tc.tile_pool(name="ps", bufs=2, space="PSUM"))

        w1a = consts.tile([P, P], F32, tag="w1a")
        w1b = consts.tile([P, P], F32, tag="w1b")
        w2a = consts.tile([P, P], F32, tag="w2a")
        w2b = consts.tile([P, P], F32, tag="w2b")
        wout = consts.tile([P, 64], F32, tag="wout")
        b1 = consts.tile([P, 1], F32, tag="b1")
        b2 = consts.tile([P, 1], F32, tag="b2")
        bout = consts.tile([64, 1], F32, tag="bout")
        ident = consts.tile([P, P], F32, tag="ident")
        xot = consts.tile([P, mp8], F32, tag="xot")
        g1 = consts.tile([P, n_tiles1 * S], I32, tag="g1")
        g2 = consts.tile([P, SELF_TILES * (S + 1)], I32, tag="g2")
        for t_sb, t_d in ((w1a, w1a_d), (w1b, w1b_d), (w2a, w2a_d), (w2b, w2b_d),
                          (wout, wout_d), (b1, b1_d), (b2, b2_d), (bout, bout_d),
                          (xot, xot_d), (g1, g1_d), (g2, g2_d)):
            nc.sync.dma_start(out=t_sb[:], in_=t_d[:, :])
        make_identity(nc, ident[:])

        def neigh_aggT(wide, n_slots):
            """max-agg the gathered neighbor rows and return aggT (feature-
            major) in SBUF."""
            agg = sb_pool.tile([P, D], F32, tag="agg")
            wide3 = wide[:].rearrange("p (s f) -> p f s", f=D)
            nc.vector.reduce_max(agg[:], wide3[:, :, (n_slots - S):],
                                 axis=mybir.AxisListType.X)
            aggT_ps = ps.tile([P, P], F32, tag="aggT_ps", space="PSUM")
            nc.tensor.transpose(aggT_ps[:], agg[:], ident[:])
            aggT = sb_pool.tile([P, P], F32, tag="aggT")
            nc.scalar.copy(out=aggT[:], in_=aggT_ps[:])
            return aggT

        def h_from(ownT_ap, aggT, wa, wb, bias, act_fn):
            hps = ps.tile([P, P], F32, tag="hps", space="PSUM")
            nc.tensor.matmul(hps[:], lhsT=wa[:], rhs=ownT_ap, start=True,
                             stop=False)
            nc.tensor.matmul(hps[:], lhsT=wb[:], rhs=aggT[:], start=False,
                             stop=True)
            hT = sb_pool.tile([P, P], F32, tag="hT")
            nc.scalar.activation(hT[:], hps[:], act_fn, bias=bias[:, :1])
            return hT

        # ---- phase 1: h1 for this core's shard (own rows from xot) ----
        ci = 0
        for t in range(n_tiles1):
            wide = wide_pool.tile([P, S * D], F32, tag="wide")
            for s in range(S):
                nc.gpsimd.indirect_dma_start(
                    out=wide[:, s * D:(s + 1) * D],
                    out_offset=None,
                    in_=x_d[:, :],
                    in_offset=bass.IndirectOffsetOnAxis(
                        ap=g1[:, t * S + s:t * S + s + 1], axis=0),
                )
            aggT = neigh_aggT(wide, S)
            hT = h_from(xot[:, t * P:(t + 1) * P], aggT, w1a, w1b, b1,
                        mybir.ActivationFunctionType.Relu)
            nm_ps = ps.tile([P, P], F32, tag="nm_ps", space="PSUM")
            nc.tensor.transpose(nm_ps[:], hT[:], ident[:])
            h1nm = sb_pool.tile([P, P], F32, tag="h1nm")
            nc.vector.tensor_copy(out=h1nm[:], in_=nm_ps[:])
            nc.sync.dma_start(out=h1_mine.ap()[t * P:(t + 1) * P, :],
                              in_=h1nm[:])
            if t + 1 == cb[ci + 1]:
                r0, r1 = cb[ci] * P, cb[ci + 1] * P
                nc.gpsimd.collective_compute(
                    "AllGather",
                    mybir.AluOpType.bypass,
                    replica_groups=[list(range(N_CORES))],
                    ins=[h1_mine.ap()[r0:r1, :].opt()],
                    outs=[h1_full.ap()[r0 * N_CORES:r1 * N_CORES, :].opt()],
                )
                ci += 1

        # ---- phase 2: layer 2 + projection for this core's batch rows ----
        for g in range(SELF_TILES):
            wide = wide_pool.tile([P, (S + 1) * D], F32, tag="wide2")
            for s in range(S + 1):
                nc.gpsimd.indirect_dma_start(
                    out=wide[:, s * D:(s + 1) * D],
                    out_offset=None,
                    in_=h1_full.ap()[:, :],
                    in_offset=bass.IndirectOffsetOnAxis(
                        ap=g2[:, g * (S + 1) + s:g * (S + 1) + s + 1], axis=0),
                )
            aggT = neigh_aggT(wide, S + 1)
            ownT_ps = ps.tile([P, P], F32, tag="nm_ps", space="PSUM")
            nc.tensor.transpose(ownT_ps[:], wide[:, 0:D], ident[:])
            ownT = sb_pool.tile([P, P], F32, tag="ownT")
            nc.scalar.copy(out=ownT[:], in_=ownT_ps[:])
            h2T = h_from(ownT[:], aggT, w2a, w2b, b2,
                         mybir.ActivationFunctionType.Identity)
            ops = ps.tile([64, P], F32, tag="hps", space="PSUM")
            nc.tensor.matmul(ops[:], lhsT=wout[:], rhs=h2T[:], start=True,
                             stop=True)
            outT = sb_pool.tile([64, P], F32, tag="outT")
            nc.scalar.activation(outT[:], ops[:],
                                 mybir.ActivationFunctionType.Identity,
                                 bias=bout[:, :1])
            trps = ps.tile([P, 64], F32, tag="aggT_ps", space="PSUM")
            nc.tensor.transpose(trps[:], outT[:], ident[:64, :64])
            outsb = sb_pool.tile([P, 64], F32, tag="outsb")
            nc.vector.tensor_copy(out=outsb[:], in_=trps[:])
            nc.sync.dma_start(out=out_d[g * P:(g + 1) * P, :], in_=outsb[:])

    return nc


_CACHE = {}


def _get_compiled(n_tiles1):
    if _CACHE.get("n_tiles1") != n_tiles1:
        nc = build_program(n_tiles1)
        nc.compile()
        _CACHE["nc"] = nc
        _CACHE["n_tiles1"] = n_tiles1
    return _CACHE["nc"]


def make_in_maps(x, neigh_idx, batch, W1, b1, W2, b2, Wout, bout):
    x = np.ascontiguousarray(np.asarray(x, dtype=np.float32))
    ni = np.asarray(neigh_idx, dtype=np.int64)
    bt = np.asarray(batch, dtype=np.int64)
    W1 = np.asarray(W1, dtype=np.float32)
    W2 = np.asarray(W2, dtype=np.float32)

    ref_nodes = np.unique(np.concatenate([bt, ni[bt].ravel()]))
    M = len(ref_nodes)
    mp8 = -(-M // (N_CORES * P)) * P
    mp = mp8 * N_CORES
    n_tiles1 = mp8 // P
    padded = np.concatenate([ref_nodes,
                             np.zeros(mp - M, dtype=ref_nodes.dtype)])

    cb = np.array(_chunk_tiles(n_tiles1)) * P       # row chunk boundaries
    pos = np.arange(mp)
    c, r = pos // mp8, pos % mp8
    k = np.searchsorted(cb, r, side="right") - 1     # chunk of each row
    szk = (cb[k + 1] - cb[k])
    full_pos = N_CORES * cb[k] + c * szk + (r - cb[k])
    glob2loc = np.zeros(N_NODES, dtype=np.int64)
    glob2loc[ref_nodes] = full_pos[:M]

    common = {
        "x": x,
        "w1a": np.ascontiguousarray(W1[:P]),
        "w1b": np.ascontiguousarray(W1[P:]),
        "w2a": np.ascontiguousarray(W2[:P]),
        "w2b": np.ascontiguousarray(W2[P:]),
        "wout": np.ascontiguousarray(np.asarray(Wout, np.float32)),
        "b1": np.ascontiguousarray(np.asarray(b1, np.float32).reshape(P, 1)),
        "b2": np.ascontiguousarray(np.asarray(b2, np.float32).reshape(P, 1)),
        "bout": np.ascontiguousarray(np.asarray(bout, np.float32).reshape(64, 1)),
    }
    in_maps = []
    for cc in range(N_CORES):
        own1 = padded[cc * mp8:(cc + 1) * mp8]
        # xot: this core's own-node features, feature-major [128, mp8]
        xot = np.ascontiguousarray(x[own1].T)
        # g1: [128, T*S] with g1[p, t*S+s] = ni[own1[t*128+p], s]
        n1 = ni[own1].reshape(n_tiles1, P, S).transpose(1, 0, 2)
        g1 = np.ascontiguousarray(n1.reshape(P, n_tiles1 * S).astype(np.int32))
        bc = bt[cc * NB:(cc + 1) * NB]
        l2 = np.concatenate([glob2loc[bc][:, None], glob2loc[ni[bc]]], axis=1)
        l2 = l2.reshape(SELF_TILES, P, S + 1).transpose(1, 0, 2)
        g2 = np.ascontiguousarray(
            l2.reshape(P, SELF_TILES * (S + 1)).astype(np.int32))
        in_maps.append(dict(common, xot=xot, g1=g1, g2=g2))
    return in_maps, n_tiles1


def run(in_maps, n_tiles1, trace=False, **kw):
    nc = _get_compiled(n_tiles1)
    return run_bass_kernel_spmd(nc, in_maps, core_ids=list(range(N_CORES)),
                                trace=trace, **kw)


def kernel(x, neigh_idx, batch, W1, b1, W2, b2, Wout, bout):
    in_maps, n_tiles1 = make_in_maps(x, neigh_idx, batch, W1, b1, W2, b2,
                                     Wout, bout)
    res = run(in_maps, n_tiles1)
    outs = [np.asarray(res.results[c]["out"]) for c in range(N_CORES)]
    return np.concatenate(outs, axis=0).astype(np.float32)


# revision 13
# speedup vs baseline: 1.0470x; 1.0113x over previous
"""GraphSAGE on 8 TRN2 cores — sharded-h1, per-slot gathers, Q7-lean.

Structure (sharded-h1): layer-1 h1 is computed once globally at the ~75K
nodes referenced by the output, sharded over 8 cores; shards are
AllGathered into a Shared-scratchpad h1 table; each core then runs layer 2
+ projection for its 1024 batch rows.

HW facts driving this version (probed on-device):
- An indirect DMA consumes exactly ONE offset per output partition; larger
  offset APs silently gather consecutive rows (v2's bug). So neighbor
  gathers are per-(tile,slot) instructions of 128 rows each, and the
  binding resource is the Q7 SWDGE fixed cost (~0.93us/instruction).
- dma_gather (int16) cannot address the 100K/77K-row tables.
Therefore the wins here are instruction-count and tail wins:
- own-node features come in pre-transposed per-core shards (xot input,
  the data-parallel shard of x), killing 1-of-17 gather instructions and
  the per-tile own transpose+copy;
- gather index tables are preloaded to SBUF once;
- h1_full is a per-core (Local) replica: a Shared-scratchpad variant
  concentrated all 8 cores' phase-2 gather reads on one HBM region and
  ran phase-2 gathers ~5x slower. The AllGather is chunked geometrically
  (50/30/17/3% of rows) so all but the last ~3% overlaps phase-1 work.
"""

import numpy as np

import concourse.bacc as bacc
import concourse.bass as bass
import concourse.mybir as mybir
import concourse.tile as tile
from concourse.bass_utils import run_bass_kernel_spmd
from concourse.masks import make_identity
from contextlib import ExitStack

N_NODES = 100000
D = 128
S = 16
BATCH = 8192
N_CORES = 8
NB = BATCH // N_CORES            # 1024 batch rows per core
P = 128
SELF_TILES = NB // P             # 8 phase-2 tiles per core
CHUNK_FRACS = (0.55, 0.8, 0.93, 0.985)  # AllGather chunk fracs

F32 = mybir.dt.float32
I32 = mybir.dt.int32


def _chunk_tiles(n_tiles1):
    cb = [0] + [max(1, round(f * n_tiles1)) for f in CHUNK_FRACS] + [n_tiles1]
    # ensure strictly increasing
    for i in range(1, len(cb)):
        cb[i] = max(cb[i], cb[i - 1] + 1) if cb[i - 1] + 1 <= n_tiles1 else cb[i - 1]
    cb[-1] = n_tiles1
    return sorted(set(cb))


def build_program(n_tiles1):
    mp8 = n_tiles1 * P
    mp = mp8 * N_CORES
    cb = _chunk_tiles(n_tiles1)
    nc = bacc.Bacc("TRN2", target_bir_lowering=False, debug=False,
                   enable_asserts=False, num_devices=N_CORES)

    x_d = nc.dram_tensor("x", [N_NODES, D], F32, kind="ExternalInput").ap()
    xot_d = nc.dram_tensor("xot", [P, mp8], F32, kind="ExternalInput").ap()
    g1_d = nc.dram_tensor("g1", [P, n_tiles1 * S], I32,
                          kind="ExternalInput").ap()
    g2_d = nc.dram_tensor("g2", [P, SELF_TILES * (S + 1)], I32,
                          kind="ExternalInput").ap()
    w1a_d = nc.dram_tensor("w1a", [P, P], F32, kind="ExternalInput").ap()
    w1b_d = nc.dram_tensor("w1b", [P, P], F32, kind="ExternalInput").ap()
    w2a_d = nc.dram_tensor("w2a", [P, P], F32, kind="ExternalInput").ap()
    w2b_d = nc.dram_tensor("w2b", [P, P], F32, kind="ExternalInput").ap()
    wout_d = nc.dram_tensor("wout", [P, 64], F32, kind="ExternalInput").ap()
    b1_d = nc.dram_tensor("b1", [P, 1], F32, kind="ExternalInput").ap()
    b2_d = nc.dram_tensor("b2", [P, 1], F32, kind="ExternalInput").ap()
    bout_d = nc.dram_tensor("bout", [64, 1], F32, kind="ExternalInput").ap()
    out_d = nc.dram_tensor("out", [NB, 64], F32, kind="ExternalOutput").ap()

    h1_mine = nc.dram_tensor("h1_mine", [mp8, D], F32)
    h1_full = nc.dram_tensor("h1_full", [mp, D], F32)

    with tile.TileContext(nc) as tc, ExitStack() as ctx:
        consts = ctx.enter_context(tc.tile_pool(name="consts", bufs=1))
        wide_pool = ctx.enter_context(tc.tile_pool(name="wide", bufs=7))
        sb_pool = ctx.enter_context(tc.tile_pool(name="sb", bufs=3))
        ps = ctx.enter_context(tc.tile_pool(name="ps", bufs=2, space="PSUM"))

        w1a = consts.tile([P, P], F32, tag="w1a")
        w1b = consts.tile([P, P], F32, tag="w1b")
        w2a = consts.tile([P, P], F32, tag="w2a")
        w2b = consts.tile([P, P], F32, tag="w2b")
        wout = consts.tile([P, 64], F32, tag="wout")
        b1 = consts.tile([P, 1], F32, tag="b1")
        b2 = consts.tile([P, 1], F32, tag="b2")
        bout = consts.tile([64, 1], F32, tag="bout")
        ident = consts.tile([P, P], F32, tag="ident")
        xot = consts.tile([P, mp8], F32, tag="xot")
        g1 = consts.tile([P, n_tiles1 * S], I32, tag="g1")
        g2 = consts.tile([P, SELF_TILES * (S + 1)], I32, tag="g2")
        for t_sb, t_d in ((w1a, w1a_d), (w1b, w1b_d), (w2a, w2a_d), (w2b, w2b_d),
                          (wout, wout_d), (b1, b1_d), (b2, b2_d), (bout, bout_d),
                          (xot, xot_d), (g1, g1_d), (g2, g2_d)):
            nc.sync.dma_start(out=t_sb[:], in_=t_d[:, :])
        make_identity(nc, ident[:])

        def neigh_aggT(wide, n_slots):
            """max-agg the gathered neighbor rows and return aggT (feature-
            major) in SBUF."""
            agg = sb_pool.tile([P, D], F32, tag="agg")
            wide3 = wide[:].rearrange("p (s f) -> p f s", f=D)
            nc.vector.reduce_max(agg[:], wide3[:, :, (n_slots - S):],
                                 axis=mybir.AxisListType.X)
            aggT_ps = ps.tile([P, P], F32, tag="aggT_ps", space="PSUM")
            nc.tensor.transpose(aggT_ps[:], agg[:], ident[:])
            aggT = sb_pool.tile([P, P], F32, tag="aggT")
            nc.scalar.copy(out=aggT[:], in_=aggT_ps[:])
            return aggT

        def h_from(ownT_ap, aggT, wa, wb, bias, act_fn):
            hps = ps.tile([P, P], F32, tag="hps", space="PSUM")
            nc.tensor.matmul(hps[:], lhsT=wa[:], rhs=ownT_ap, start=True,
                             stop=False)
            nc.tensor.matmul(hps[:], lhsT=wb[:], rhs=aggT[:], start=False,
                             stop=True)
            hT = sb_pool.tile([P, P], F32, tag="hT")
            nc.scalar.activation(hT[:], hps[:], act_fn, bias=bias[:, :1])
            return hT

        # ---- phase 1: h1 for this core's shard (own rows from xot) ----
        ci = 0
        for t in range(n_tiles1):
            wide = wide_pool.tile([P, S * D], F32, tag="wide")
            for s in range(S):
                nc.gpsimd.indirect_dma_start(
                    out=wide[:, s * D:(s + 1) * D],
                    out_offset=None,
                    in_=x_d[:, :],
                    in_offset=bass.IndirectOffsetOnAxis(
                        ap=g1[:, t * S + s:t * S + s + 1], axis=0),
                )
            aggT = neigh_aggT(wide, S)
            hT = h_from(xot[:, t * P:(t + 1) * P], aggT, w1a, w1b, b1,
                        mybir.ActivationFunctionType.Relu)
            nm_ps = ps.tile([P, P], F32, tag="nm_ps", space="PSUM")
            nc.tensor.transpose(nm_ps[:], hT[:], ident[:])
            h1nm = sb_pool.tile([P, P], F32, tag="h1nm")
            nc.vector.tensor_copy(out=h1nm[:], in_=nm_ps[:])
            nc.sync.dma_start(out=h1_mine.ap()[t * P:(t + 1) * P, :],
                              in_=h1nm[:])
            if t + 1 == cb[ci + 1]:
                r0, r1 = cb[ci] * P, cb[ci + 1] * P
                nc.gpsimd.collective_compute(
                    "AllGather",
                    mybir.AluOpType.bypass,
                    replica_groups=[list(range(N_CORES))],
                    ins=[h1_mine.ap()[r0:r1, :].opt()],
                    outs=[h1_full.ap()[r0 * N_CORES:r1 * N_CORES, :].opt()],
                )
                ci += 1

        # ---- phase 2: layer 2 + projection for this core's batch rows ----
        for g in range(SELF_TILES):
            wide = wide_pool.tile([P, (S + 1) * D], F32, tag="wide2")
            for s in range(S + 1):
                nc.gpsimd.indirect_dma_start(
                    out=wide[:, s * D:(s + 1) * D],
                    out_offset=None,
                    in_=h1_full.ap()[:, :],
                    in_offset=bass.IndirectOffsetOnAxis(
                        ap=g2[:, g * (S + 1) + s:g * (S + 1) + s + 1], axis=0),
                )
            aggT = neigh_aggT(wide, S + 1)
            ownT_ps = ps.tile([P, P], F32, tag="nm_ps", space="PSUM")
            nc.tensor.transpose(ownT_ps[:], wide[:, 0:D], ident[:])
            ownT = sb_pool.tile([P, P], F32, tag="ownT")
            nc.scalar.copy(out=ownT[:], in_=ownT_ps[:])
            h2T = h_from(ownT[:], aggT, w2a, w2b, b2,
                         mybir.ActivationFunctionType.Identity)
            ops = ps.tile([64, P], F32, tag="hps", space="PSUM")
            nc.tensor.matmul(ops[:], lhsT=wout[:], rhs=h2T[:], start=True,
                             stop=True)
            outT = sb_pool.tile([64, P], F32, tag="outT")
            nc.scalar.activation(outT[:], ops[:],
                                 mybir.ActivationFunctionType.Identity,
                                 bias=bout[:, :1])
            trps = ps.tile([P, 64], F32, tag="aggT_ps", space="PSUM")
            nc.tensor.transpose(trps[:], outT[:], ident[:64, :64])
            outsb = sb_pool.tile([P, 64], F32, tag="outsb")
            nc.vector.tensor_copy(out=outsb[:], in_=trps[:])
            nc.sync.dma_start(out=out_d[g * P:(g + 1) * P, :], in_=outsb[:])

    return nc


_CACHE = {}


def _get_compiled(n_tiles1):
    if _CACHE.get("n_tiles1") != n_tiles1:
        nc = build_program(n_tiles1)
        nc.compile()
        _CACHE["nc"] = nc
        _CACHE["n_tiles1"] = n_tiles1
    return _CACHE["nc"]


def make_in_maps(x, neigh_idx, batch, W1, b1, W2, b2, Wout, bout):
    x = np.ascontiguousarray(np.asarray(x, dtype=np.float32))
    ni = np.asarray(neigh_idx, dtype=np.int64)
    bt = np.asarray(batch, dtype=np.int64)
    W1 = np.asarray(W1, dtype=np.float32)
    W2 = np.asarray(W2, dtype=np.float32)

    ref_nodes = np.unique(np.concatenate([bt, ni[bt].ravel()]))
    M = len(ref_nodes)
    mp8 = -(-M // (N_CORES * P)) * P
    mp = mp8 * N_CORES
    n_tiles1 = mp8 // P
    padded = np.concatenate([ref_nodes,
                             np.zeros(mp - M, dtype=ref_nodes.dtype)])

    cb = np.array(_chunk_tiles(n_tiles1)) * P       # row chunk boundaries
    pos = np.arange(mp)
    c, r = pos // mp8, pos % mp8
    k = np.searchsorted(cb, r, side="right") - 1     # chunk of each row
    szk = (cb[k + 1] - cb[k])
    full_pos = N_CORES * cb[k] + c * szk + (r - cb[k])
    glob2loc = np.zeros(N_NODES, dtype=np.int64)
    glob2loc[ref_nodes] = full_pos[:M]

    common = {
        "x": x,
        "w1a": np.ascontiguousarray(W1[:P]),
        "w1b": np.ascontiguousarray(W1[P:]),
        "w2a": np.ascontiguousarray(W2[:P]),
        "w2b": np.ascontiguousarray(W2[P:]),
        "wout": np.ascontiguousarray(np.asarray(Wout, np.float32)),
        "b1": np.ascontiguousarray(np.asarray(b1, np.float32).reshape(P, 1)),
        "b2": np.ascontiguousarray(np.asarray(b2, np.float32).reshape(P, 1)),
        "bout": np.ascontiguousarray(np.asarray(bout, np.float32).reshape(64, 1)),
    }
    in_maps = []
    for cc in range(N_CORES):
        own1 = padded[cc * mp8:(cc + 1) * mp8]
        # xot: this core's own-node features, feature-major [128, mp8]
        xot = np.ascontiguousarray(x[own1].T)
        # g1: [128, T*S] with g1[p, t*S+s] = ni[own1[t*128+p], s]
        n1 = ni[own1].reshape(n_tiles1, P, S).transpose(1, 0, 2)
        g1 = np.ascontiguousarray(n1.reshape(P, n_tiles1 * S).astype(np.int32))
        bc = bt[cc * NB:(cc + 1) * NB]
        l2 = np.concatenate([glob2loc[bc][:, None], glob2loc[ni[bc]]], axis=1)
        l2 = l2.reshape(SELF_TILES, P, S + 1).transpose(1, 0, 2)
        g2 = np.ascontiguousarray(
            l2.reshape(P, SELF_TILES * (S + 1)).astype(np.int32))
        in_maps.append(dict(common, xot=xot, g1=g1, g2=g2))
    return in_maps, n_tiles1


def run(in_maps, n_tiles1, trace=False, **kw):
    nc = _get_compiled(n_tiles1)
    return run_bass_kernel_spmd(nc, in_maps, core_ids=list(range(N_CORES)),
                                trace=trace, **kw)


def kernel(x, neigh_idx, batch, W1, b1, W2, b2, Wout, bout):
    in_maps, n_tiles1 = make_in_maps(x, neigh_idx, batch, W1, b1, W2, b2,
                                     Wout, bout)
    res = run(in_maps, n_tiles1)
    outs = [np.asarray(res.results[c]["out"]) for c in range(N_CORES)]
    return np.concatenate(outs, axis=0).astype(np.float32)
